# revision 11
# baseline (speedup 1.0000x reference)
"""AttentionHead kernel for Trainium2, 8 NeuronCores.

Problem: x:(4,4096,1024) f32, W_qkv:(1024,192) f32, attn_mask:(4,4096) bool.
  qkv = x @ W_qkv ; q,k,v = split(qkv) ; scores = q k^T / 8 (masked keys -> -inf)
  out = softmax(scores) @ v   -> (4, 4096, 64) f32

Sharding: 8 cores = (batch b, query-half h). Each core receives x[b] rolled so
its 2048 queries are rows 0:2048, computes k/v over all 4096 (rolled) keys, and
attention for its query half. Key order is a permutation, which softmax+PV is
invariant to as long as the mask is permuted identically.

Per-core pipeline (all matmuls bf16 with fp32 PSUM accumulation):
  1. x tiles -> SBUF f32 (HWDGE), cast bf16 (GPSIMD), PE-transpose -> x^T tiles
  2. qkv^T = W^T-stationary matmuls -> q^T,k^T,v^T [64, L] bf16 in SBUF
  3. v^T PE-transposed back to v_aug [keys,65] per 128-key chunk (col 64 = 1.0)
  4. per 1024-query group, per 128-key chunk:
       s^T = k^T-chunk^T q^T   (PSUM f32 [128 keys, 1024 q])
       e^T = exp(0.125*s^T + mask_bias[key])  (ACT, -> SBUF bf16)
       pv[qt] += e^T-slice^T @ v_aug-chunk    (PSUM f32 [128 q, 65])
     pv col 64 accumulates sum(e) -> out = pv[:, :64] * (1/pv[:, 64])
"""

import numpy as np

import concourse.bass as bass
import concourse.mybir as mybir
import concourse.tile as tile
from concourse import bacc
from concourse.bass_utils import run_bass_kernel_spmd
from concourse.masks import make_identity

B, L, D = 4, 4096, 1024
HS = 64          # head size
LQ = L // 2      # queries per core
N_CORES = 8
MASK_NEG = -30000.0

F32 = mybir.dt.float32
BF16 = mybir.dt.bfloat16


def build_module():
    nc = bacc.Bacc("TRN2", target_bir_lowering=False, debug=False,
                   num_devices=N_CORES)
    x_ap = nc.dram_tensor("x", [L, D], F32, kind="ExternalInput").ap()
    w_ap = nc.dram_tensor("w", [D, 3 * HS], F32, kind="ExternalInput").ap()
    mb_ap = nc.dram_tensor("mb", [128, L // 128], F32, kind="ExternalInput").ap()
    out_ap = nc.dram_tensor("out", [LQ, HS], F32, kind="ExternalOutput").ap()

    with tile.TileContext(nc) as tc:
        _build_kernel(tc, x_ap, w_ap, mb_ap, out_ap)
    nc.compile()
    return nc


def _build_kernel(tc, x_ap, w_ap, mb_ap, out_ap, dbg=None):
    from contextlib import ExitStack
    with ExitStack() as ctx:
        _build_kernel_inner(tc, ctx, x_ap, w_ap, mb_ap, out_ap, dbg)


def _build_kernel_inner(tc, ctx, x_ap, w_ap, mb_ap, out_ap, dbg=None):
    nc = tc.nc
    DC = D // 128          # 8 d-chunks
    NLG = L // 512         # 8 l-groups of 512 rows
    NQG = LQ // 1024       # 2 query groups
    NKC = L // 128         # 32 key chunks
    W3 = 3 * HS            # 192

    const = ctx.enter_context(tc.tile_pool(name="const", bufs=1))
    xf_pool = ctx.enter_context(tc.tile_pool(name="xf", bufs=3))
    xb_pool = ctx.enter_context(tc.tile_pool(name="xb", bufs=6))
    xt_pool = ctx.enter_context(tc.tile_pool(name="xt", bufs=10))
    e_pool = ctx.enter_context(tc.tile_pool(name="e", bufs=3))
    o_pool = ctx.enter_context(tc.tile_pool(name="o", bufs=3))
    # PSUM: sp (2 banks x 2) shared by x^T-transpose stage and scores stage;
    # qp (1 bank x 2) qkv accum + v_aug transposes; pv (2 banks x 2).
    sp_pool = ctx.enter_context(tc.tile_pool(name="sp", bufs=2, space="PSUM"))
    qp_pool = ctx.enter_context(tc.tile_pool(name="qp", bufs=2, space="PSUM"))
    pv_pool = ctx.enter_context(tc.tile_pool(name="pv", bufs=1, space="PSUM"))

    # ---- constants ----
    wf = const.tile([128, DC * W3], F32)
    for dc in range(DC):
        nc.sync.dma_start(wf[:, dc * W3:(dc + 1) * W3],
                          w_ap[dc * 128:(dc + 1) * 128, :])
    wb = const.tile([128, DC * W3], BF16)
    nc.vector.tensor_copy(wb[:], wf[:])
    mbias = const.tile([128, NKC], F32)
    nc.sync.dma_start(mbias[:], mb_ap[:])
    ident = const.tile([128, 128], BF16)
    make_identity(nc, ident[:])

    qT = const.tile([64, LQ], BF16)
    kT = const.tile([64, L], BF16)
    vT = const.tile([64, L], BF16)
    vaug = const.tile([128, NKC, HS + 1], BF16)
    nc.vector.memset(vaug[:, :, HS:HS + 1], 1.0)

    # ---- phase 1+2: x -> x^T -> qkv^T ----
    for lg in range(NLG):
        xts = []
        for dc in range(DC):
            xtp = sp_pool.tile([128, 512], BF16, tag="sp")
            xts.append(xtp)
        xbs = []
        for lt in range(4):
            xf = xf_pool.tile([128, D], F32)
            nc.sync.dma_start(xf[:], x_ap[lg * 512 + lt * 128:
                                          lg * 512 + (lt + 1) * 128, :])
            xb = xb_pool.tile([128, D], BF16)
            nc.gpsimd.tensor_copy(xb[:], xf[:])
            xbs.append(xb)
        xt_sb = []
        for dc in range(DC):
            xtp = xts[dc]
            for lt in range(4):
                nc.tensor.transpose(xtp[:, lt * 128:(lt + 1) * 128],
                                    xbs[lt][:, dc * 128:(dc + 1) * 128],
                                    ident[:])
            xt = xt_pool.tile([128, 512], BF16)
            nc.vector.tensor_copy(xt[:], xtp[:])
            xt_sb.append(xt)

        if lg < NLG // 2:
            # own query half: need q, k, v
            qk_ps = qp_pool.tile([128, 512], F32, tag="qp")
            v_ps = qp_pool.tile([64, 512], F32, tag="qp")
            for dc in range(DC):
                nc.tensor.matmul(qk_ps[:], lhsT=wb[:, dc * W3: dc * W3 + 128],
                                 rhs=xt_sb[dc][:],
                                 start=(dc == 0), stop=(dc == DC - 1))
            for dc in range(DC):
                nc.tensor.matmul(v_ps[:], lhsT=wb[:, dc * W3 + 128: dc * W3 + 192],
                                 rhs=xt_sb[dc][:],
                                 start=(dc == 0), stop=(dc == DC - 1))
            sl = slice(lg * 512, (lg + 1) * 512)
            nc.vector.tensor_copy(qT[:, sl], qk_ps[0:64, :])
            nc.vector.tensor_copy(kT[:, sl], qk_ps[64:128, :])
            nc.vector.tensor_copy(vT[:, sl], v_ps[:, :])
        else:
            # other half: only k, v  (W columns 64:192 -> k|v stacked)
            kv_ps = qp_pool.tile([128, 512], F32, tag="qp")
            for dc in range(DC):
                nc.tensor.matmul(kv_ps[:], lhsT=wb[:, dc * W3 + 64: dc * W3 + 192],
                                 rhs=xt_sb[dc][:],
                                 start=(dc == 0), stop=(dc == DC - 1))
            sl = slice(lg * 512, (lg + 1) * 512)
            nc.vector.tensor_copy(kT[:, sl], kv_ps[0:64, :])
            nc.vector.tensor_copy(vT[:, sl], kv_ps[64:128, :])

    # ---- phase 3: v_aug [key-chunk 128, 65] ----
    for kc in range(NKC):
        vtp = qp_pool.tile([128, 64], BF16, tag="qp")
        nc.tensor.transpose(vtp[:], vT[:, kc * 128:(kc + 1) * 128],
                            ident[0:64, 0:64])
        nc.vector.tensor_copy(vaug[:, kc, 0:HS], vtp[:])

    if dbg is not None:
        nc.gpsimd.dma_start(dbg["qT"][:], qT[:])
        nc.gpsimd.dma_start(dbg["kT"][:], kT[:])
        nc.gpsimd.dma_start(dbg["vT"][:], vT[:])
        nc.gpsimd.dma_start(dbg["vaug"][:], vaug[:].rearrange("p a b -> p (a b)"))

    # ---- phase 4: attention ----
    for qg in range(NQG):
        # pv bank layout: qt 0-3 in bank 0 at qt*65, qt 4-7 in bank 1.
        pv = pv_pool.tile([128, 1024], F32)
        pv_off = [(qt // 4) * 512 + (qt % 4) * 65 for qt in range(8)]
        for kc in range(NKC):
            s = sp_pool.tile([128, 1024], F32, tag="sp")
            for half in range(2):
                nc.tensor.matmul(
                    s[:, half * 512:(half + 1) * 512],
                    lhsT=kT[:, kc * 128:(kc + 1) * 128],
                    rhs=qT[:, qg * 1024 + half * 512:
                           qg * 1024 + (half + 1) * 512],
                    start=True, stop=True)
            e = e_pool.tile([128, 1024], BF16)
            nc.scalar.activation(e[:], s[:], mybir.ActivationFunctionType.Exp,
                                 bias=mbias[:, kc:kc + 1], scale=0.125)
            if dbg is not None and qg == 0 and kc == 0:
                sc = o_pool.tile([128, 1024], F32, tag="sdbg")
                nc.vector.tensor_copy(sc[:], s[:])
                nc.sync.dma_start(dbg["s0"][:], sc[:])
                nc.gpsimd.dma_start(dbg["e0"][:], e[:])
            for qt in range(8):
                # start=True clears has_written for the WHOLE bank, so only
                # the first matmul touching each pv bank may set it; the
                # other qt slices of that bank first-write via cleared bits.
                nc.tensor.matmul(pv[:, pv_off[qt]:pv_off[qt] + 65],
                                 lhsT=e[:, qt * 128:(qt + 1) * 128],
                                 rhs=vaug[:, kc, :],
                                 start=(kc == 0 and qt % 4 == 0),
                                 stop=(kc == NKC - 1),
                                 skip_group_check=True)
        if dbg is not None and qg == 0:
            pvc = o_pool.tile([128, 1024], F32, tag="sdbg")
            nc.vector.memset(pvc[:], 0.0)
            for qt in range(8):
                nc.vector.tensor_copy(pvc[:, pv_off[qt]:pv_off[qt] + 65],
                                      pv[:, pv_off[qt]:pv_off[qt] + 65])
            nc.sync.dma_start(dbg["pv0"][:], pvc[:])
        for qt in range(8):
            r = o_pool.tile([128, 1], F32, tag="r")
            nc.vector.reciprocal(r[:], pv[:, pv_off[qt] + 64:pv_off[qt] + 65])
            o = o_pool.tile([128, HS], F32, tag="o")
            nc.vector.tensor_scalar_mul(o[:], pv[:, pv_off[qt]:pv_off[qt] + 64],
                                        r[:])
            row0 = qg * 1024 + qt * 128
            nc.sync.dma_start(out_ap[row0:row0 + 128, :], o[:])


_NC_CACHE = None


def _get_module():
    global _NC_CACHE
    if _NC_CACHE is None:
        _NC_CACHE = build_module()
    return _NC_CACHE


def make_in_maps(x, attn_mask, W_qkv):
    """Host-side sharding: core (b, h) gets x[b] rolled by h*2048 rows."""
    x = np.ascontiguousarray(np.asarray(x, dtype=np.float32))
    W_qkv = np.ascontiguousarray(np.asarray(W_qkv, dtype=np.float32))
    mask = np.asarray(attn_mask)
    in_maps = []
    for b in range(B):
        for h in range(2):
            if h == 0:
                xr = x[b]
                mr = mask[b]
            else:
                xr = np.concatenate([x[b, LQ:], x[b, :LQ]], axis=0)
                mr = np.concatenate([mask[b, LQ:], mask[b, :LQ]], axis=0)
            mb = np.where(mr, 0.0, MASK_NEG).astype(np.float32)
            mb = np.ascontiguousarray(mb.reshape(L // 128, 128).T)
            in_maps.append({"x": np.ascontiguousarray(xr),
                            "w": W_qkv, "mb": mb})
    return in_maps


def assemble_out(results):
    out = np.empty((B, L, HS), dtype=np.float32)
    for b in range(B):
        for h in range(2):
            out[b, h * LQ:(h + 1) * LQ] = results[b * 2 + h]["out"]
    return out


def kernel(x, attn_mask, W_qkv):
    nc = _get_module()
    in_maps = make_in_maps(x, attn_mask, W_qkv)
    res = run_bass_kernel_spmd(nc, in_maps, core_ids=list(range(N_CORES)))
    return assemble_out(res.results)


# revision 32
# speedup vs baseline: 12676.0201x; 12676.0201x over previous
"""AttentionHead kernel for Trainium2, 8 NeuronCores.

Problem: x:(4,4096,1024) f32, W_qkv:(1024,192) f32, attn_mask:(4,4096) bool.
  qkv = x @ W_qkv ; q,k,v = split(qkv) ; scores = q k^T / 8 (masked keys -> -inf)
  out = softmax(scores) @ v   -> (4, 4096, 64) f32

Sharding: 8 cores = (batch b, query-half h). Each core receives x[b] rolled so
its 2048 queries are rows 0:2048, computes k/v over all 4096 (rolled) keys, and
attention for its query half. Key order is a permutation, which softmax+PV is
invariant to as long as the mask is permuted identically.

Per-core pipeline (all matmuls bf16 with fp32 PSUM accumulation):
  1. x tiles -> SBUF f32 (HWDGE), cast bf16 (GPSIMD), PE-transpose -> x^T tiles
  2. qkv^T = W^T-stationary matmuls -> q^T,k^T,v^T [64, L] bf16 in SBUF
  3. v^T PE-transposed back to v_aug [keys,65] per 128-key chunk (col 64 = 1.0)
  4. per 1024-query group, per 128-key chunk:
       s^T = k^T-chunk^T q^T   (PSUM f32 [128 keys, 1024 q])
       e^T = exp(0.125*s^T + mask_bias[key])  (ACT, -> SBUF bf16)
       pv[qt] += e^T-slice^T @ v_aug-chunk    (PSUM f32 [128 q, 65])
     pv col 64 accumulates sum(e) -> out = pv[:, :64] * (1/pv[:, 64])
"""

import numpy as np

import concourse.bass as bass
import concourse.mybir as mybir
import concourse.tile as tile
from concourse import bacc
from concourse.bass_utils import run_bass_kernel_spmd
from concourse.masks import make_identity

B, L, D = 4, 4096, 1024
HS = 64          # head size
LQ = L // 2      # queries per core
N_CORES = 8
MASK_NEG = -30000.0

F32 = mybir.dt.float32
BF16 = mybir.dt.bfloat16


def build_module(bench_iters=None):
    nc = bacc.Bacc("TRN2", target_bir_lowering=False, debug=False,
                   num_devices=N_CORES)
    x_ap = nc.dram_tensor("x", [L, D], BF16, kind="ExternalInput").ap()
    w_ap = nc.dram_tensor("w", [D, 3 * HS], F32, kind="ExternalInput").ap()
    mb_ap = nc.dram_tensor("mb", [128, L // 128], F32, kind="ExternalInput").ap()
    out_ap = nc.dram_tensor("out", [LQ, HS], F32, kind="ExternalOutput").ap()

    with tile.TileContext(nc) as tc:
        _build_kernel(tc, x_ap, w_ap, mb_ap, out_ap, bench_iters=bench_iters)
    nc.compile()
    return nc


VARIANT = {"cast_dma": False, "prep_only": False}


def _build_kernel(tc, x_ap, w_ap, mb_ap, out_ap, dbg=None, bench_iters=None):
    from contextlib import ExitStack
    with ExitStack() as ctx:
        _build_kernel_inner(tc, ctx, x_ap, w_ap, mb_ap, out_ap, dbg,
                            bench_iters)


def _build_kernel_inner(tc, ctx, x_ap, w_ap, mb_ap, out_ap, dbg=None,
                        bench_iters=None):
    nc = tc.nc
    DC = D // 128          # 8 d-chunks
    NLG = L // 512         # 8 l-groups of 512 rows
    NQG = LQ // 1024       # 2 query groups
    NKC = L // 128         # 32 key chunks
    W3 = 3 * HS            # 192

    const = ctx.enter_context(tc.tile_pool(name="const", bufs=1))
    xf_pool = ctx.enter_context(tc.tile_pool(name="xf", bufs=3))
    xb_pool = ctx.enter_context(tc.tile_pool(name="xb", bufs=8))
    xt_pool = ctx.enter_context(tc.tile_pool(name="xt", bufs=10))
    e_pool = ctx.enter_context(tc.tile_pool(name="e", bufs=4))
    o_pool = ctx.enter_context(tc.tile_pool(name="o", bufs=3))
    # PSUM: sp (2 banks x 2) shared by x^T-transpose stage and scores stage;
    # qp (1 bank x 2) qkv accum + v_aug transposes; pv (2 banks x 2).
    sp_pool = ctx.enter_context(tc.tile_pool(name="sp", bufs=2, space="PSUM"))
    qp_pool = ctx.enter_context(tc.tile_pool(name="qp", bufs=2, space="PSUM"))
    pv_pool = ctx.enter_context(tc.tile_pool(name="pv", bufs=1, space="PSUM"))

    # ---- constants ----
    wf = const.tile([128, DC * W3], F32)
    for dc in range(DC):
        nc.sync.dma_start(wf[:, dc * W3:(dc + 1) * W3],
                          w_ap[dc * 128:(dc + 1) * 128, :])
    wb = const.tile([128, DC * W3], BF16)
    nc.vector.tensor_copy(wb[:], wf[:])
    mbias = const.tile([128, NKC], F32)
    nc.sync.dma_start(mbias[:], mb_ap[:])
    ident = const.tile([128, 128], BF16)
    make_identity(nc, ident[:])

    qT = const.tile([64, LQ], BF16)
    kT = const.tile([64, L], BF16)
    vT = const.tile([64, L], BF16)
    vaug = const.tile([128, NKC, HS + 1], BF16)
    nc.vector.memset(vaug[:, :, HS:HS + 1], 1.0)

    if bench_iters is not None:
        loop_cm = tc.For_i(0, bench_iters, 1)
        loop_cm.__enter__()

    # ---- phase 1+2: x -> x^T -> qkv^T ----
    for lg in range(NLG):
        xbs = []
        for lt in range(4):
            xb = xb_pool.tile([128, D], BF16)
            rows = slice(lg * 512 + lt * 128, lg * 512 + (lt + 1) * 128)
            nc.sync.dma_start(xb[:], x_ap[rows, :])
            xbs.append(xb)
        # transpose 2 d-chunks per PSUM tile; one wide DVE copy per pair
        xt_sb = []
        for dp in range(DC // 2):
            xtp = sp_pool.tile([128, 1024], BF16, tag="sp")
            for half in range(2):
                dc = dp * 2 + half
                for lt in range(4):
                    nc.tensor.transpose(
                        xtp[:, half * 512 + lt * 128:
                            half * 512 + (lt + 1) * 128],
                        xbs[lt][:, dc * 128:(dc + 1) * 128],
                        ident[:])
            xt = xt_pool.tile([128, 1024], BF16)
            nc.vector.tensor_copy(xt[:], xtp[:])
            xt_sb.append(xt)

        def xt_slice(dc):
            return xt_sb[dc // 2][:, (dc % 2) * 512:(dc % 2 + 1) * 512]

        if lg < NLG // 2:
            # own query half: need q, k, v
            qk_ps = qp_pool.tile([128, 512], F32, tag="qp")
            v_ps = qp_pool.tile([64, 512], F32, tag="qp")
            for dc in range(DC):
                nc.tensor.matmul(qk_ps[:], lhsT=wb[:, dc * W3: dc * W3 + 128],
                                 rhs=xt_slice(dc),
                                 start=(dc == 0), stop=(dc == DC - 1))
            for dc in range(DC):
                nc.tensor.matmul(v_ps[:], lhsT=wb[:, dc * W3 + 128: dc * W3 + 192],
                                 rhs=xt_slice(dc),
                                 start=(dc == 0), stop=(dc == DC - 1))
            sl = slice(lg * 512, (lg + 1) * 512)
            nc.vector.tensor_copy(qT[:, sl], qk_ps[0:64, :])
            nc.vector.tensor_copy(kT[:, sl], qk_ps[64:128, :])
            nc.vector.tensor_copy(vT[:, sl], v_ps[:, :])
        else:
            # other half: only k, v  (W columns 64:192 -> k|v stacked)
            kv_ps = qp_pool.tile([128, 512], F32, tag="qp")
            for dc in range(DC):
                nc.tensor.matmul(kv_ps[:], lhsT=wb[:, dc * W3 + 64: dc * W3 + 192],
                                 rhs=xt_slice(dc),
                                 start=(dc == 0), stop=(dc == DC - 1))
            sl = slice(lg * 512, (lg + 1) * 512)
            nc.vector.tensor_copy(kT[:, sl], kv_ps[0:64, :])
            nc.vector.tensor_copy(vT[:, sl], kv_ps[64:128, :])

        # v_aug chunks for this l-group (keys lg*512 .. +512)
        for kc in range(lg * 4, (lg + 1) * 4):
            vtp = qp_pool.tile([128, 64], BF16, tag="qp")
            nc.tensor.transpose(vtp[:], vT[:, kc * 128:(kc + 1) * 128],
                                ident[0:64, 0:64])
            nc.vector.tensor_copy(vaug[:, kc, 0:HS], vtp[:])

    if dbg is not None:
        nc.gpsimd.dma_start(dbg["qT"][:], qT[:])
        nc.gpsimd.dma_start(dbg["kT"][:], kT[:])
        nc.gpsimd.dma_start(dbg["vT"][:], vT[:])
        nc.gpsimd.dma_start(dbg["vaug"][:], vaug[:].rearrange("p a b -> p (a b)"))

    if VARIANT["prep_only"]:
        # diagnostic: skip attention; just flush something to out
        o = o_pool.tile([128, HS], F32, tag="o")
        nc.vector.tensor_copy(o[:], vaug[:, 0, 0:HS])
        for qt in range(LQ // 128):
            nc.sync.dma_start(out_ap[qt * 128:(qt + 1) * 128, :], o[:])
        if bench_iters is not None:
            loop_cm.__exit__(None, None, None)
        return

    # ---- phase 4: attention ----
    for qg in range(NQG):
        # pv bank layout: qt 0-3 in bank 0 at qt*65, qt 4-7 in bank 1.
        pv = pv_pool.tile([128, 1024], F32)
        pv_off = [(qt // 4) * 512 + (qt % 4) * 65 for qt in range(8)]
        for kc in range(NKC):
            s = sp_pool.tile([128, 1024], F32, tag="sp")
            for half in range(2):
                nc.tensor.matmul(
                    s[:, half * 512:(half + 1) * 512],
                    lhsT=kT[:, kc * 128:(kc + 1) * 128],
                    rhs=qT[:, qg * 1024 + half * 512:
                           qg * 1024 + (half + 1) * 512],
                    start=True, stop=True)
            e = e_pool.tile([128, 1024], BF16)
            nc.scalar.activation(e[:], s[:], mybir.ActivationFunctionType.Exp,
                                 bias=mbias[:, kc:kc + 1], scale=0.125)
            if dbg is not None and qg == 0 and kc == 0:
                sc = o_pool.tile([128, 1024], F32, tag="sdbg")
                nc.vector.tensor_copy(sc[:], s[:])
                nc.sync.dma_start(dbg["s0"][:], sc[:])
                nc.gpsimd.dma_start(dbg["e0"][:], e[:])
            for qt in range(8):
                # start=True clears has_written for the WHOLE bank, so only
                # the first matmul touching each pv bank may set it; the
                # other qt slices of that bank first-write via cleared bits.
                nc.tensor.matmul(pv[:, pv_off[qt]:pv_off[qt] + 65],
                                 lhsT=e[:, qt * 128:(qt + 1) * 128],
                                 rhs=vaug[:, kc, :],
                                 start=(kc == 0 and qt % 4 == 0),
                                 stop=(kc == NKC - 1),
                                 skip_group_check=True)
        if dbg is not None and qg == 0:
            pvc = o_pool.tile([128, 1024], F32, tag="sdbg")
            nc.vector.memset(pvc[:], 0.0)
            for qt in range(8):
                nc.vector.tensor_copy(pvc[:, pv_off[qt]:pv_off[qt] + 65],
                                      pv[:, pv_off[qt]:pv_off[qt] + 65])
            nc.sync.dma_start(dbg["pv0"][:], pvc[:])
        for qt in range(8):
            r = o_pool.tile([128, 1], F32, tag="r")
            nc.vector.reciprocal(r[:], pv[:, pv_off[qt] + 64:pv_off[qt] + 65])
            o = o_pool.tile([128, HS], F32, tag="o")
            nc.vector.tensor_scalar_mul(o[:], pv[:, pv_off[qt]:pv_off[qt] + 64],
                                        r[:])
            row0 = qg * 1024 + qt * 128
            nc.sync.dma_start(out_ap[row0:row0 + 128, :], o[:])

    if bench_iters is not None:
        loop_cm.__exit__(None, None, None)


_NC_CACHE = None


def _get_module():
    global _NC_CACHE
    if _NC_CACHE is None:
        _NC_CACHE = build_module()
    return _NC_CACHE


def make_in_maps(x, attn_mask, W_qkv):
    """Host-side sharding: core (b, h) gets x[b] rolled by h*2048 rows."""
    import ml_dtypes
    x = np.asarray(x, dtype=np.float32).astype(ml_dtypes.bfloat16)
    W_qkv = np.ascontiguousarray(np.asarray(W_qkv, dtype=np.float32))
    mask = np.asarray(attn_mask)
    in_maps = []
    for b in range(B):
        for h in range(2):
            if h == 0:
                xr = x[b]
                mr = mask[b]
            else:
                xr = np.concatenate([x[b, LQ:], x[b, :LQ]], axis=0)
                mr = np.concatenate([mask[b, LQ:], mask[b, :LQ]], axis=0)
            bias = np.where(mr, 0.0, MASK_NEG).astype(np.float32)
            mb = np.ascontiguousarray(bias.reshape(L // 128, 128).T)
            in_maps.append({"x": np.ascontiguousarray(xr),
                            "w": W_qkv, "mb": mb})
    return in_maps


def _pos2l():
    """Device position i = b*512 + j*128 + p  <->  row l = b*512 + 4p + j
    (from the 4-rows-per-partition DMA layout)."""
    b = np.arange(L // 512)[:, None, None]
    j = np.arange(4)[None, :, None]
    p = np.arange(128)[None, None, :]
    return (b * 512 + 4 * p + j).reshape(-1)


def assemble_out(results):
    out = np.empty((B, L, HS), dtype=np.float32)
    for b in range(B):
        for h in range(2):
            out[b, h * LQ:(h + 1) * LQ] = results[b * 2 + h]["out"]
    return out


def kernel(x, attn_mask, W_qkv):
    nc = _get_module()
    in_maps = make_in_maps(x, attn_mask, W_qkv)
    res = run_bass_kernel_spmd(nc, in_maps, core_ids=list(range(N_CORES)))
    return assemble_out(res.results)


# revision 36
# speedup vs baseline: 14239.0706x; 1.1233x over previous
"""AttentionHead kernel for Trainium2, 8 NeuronCores.

Problem: x:(4,4096,1024) f32, W_qkv:(1024,192) f32, attn_mask:(4,4096) bool.
  qkv = x @ W_qkv ; q,k,v = split(qkv) ; scores = q k^T / 8 (masked keys -> -inf)
  out = softmax(scores) @ v   -> (4, 4096, 64) f32

Sharding: 8 cores = (batch b, query-half h). Each core receives x[b] rolled so
its 2048 queries are rows 0:2048, computes k/v over all 4096 (rolled) keys, and
attention for its query half. Key order is a permutation, which softmax+PV is
invariant to as long as the mask is permuted identically.

Per-core pipeline (all matmuls bf16 with fp32 PSUM accumulation):
  1. x tiles -> SBUF f32 (HWDGE), cast bf16 (GPSIMD), PE-transpose -> x^T tiles
  2. qkv^T = W^T-stationary matmuls -> q^T,k^T,v^T [64, L] bf16 in SBUF
  3. v^T PE-transposed back to v_aug [keys,65] per 128-key chunk (col 64 = 1.0)
  4. per 1024-query group, per 128-key chunk:
       s^T = k^T-chunk^T q^T   (PSUM f32 [128 keys, 1024 q])
       e^T = exp(0.125*s^T + mask_bias[key])  (ACT, -> SBUF bf16)
       pv[qt] += e^T-slice^T @ v_aug-chunk    (PSUM f32 [128 q, 65])
     pv col 64 accumulates sum(e) -> out = pv[:, :64] * (1/pv[:, 64])
"""

import numpy as np

import concourse.bass as bass
import concourse.mybir as mybir
import concourse.tile as tile
from concourse import bacc
from concourse.bass_utils import run_bass_kernel_spmd
from concourse.masks import make_identity

B, L, D = 4, 4096, 1024
HS = 64          # head size
LQ = L // 2      # queries per core
N_CORES = 8
MASK_NEG = -30000.0

F32 = mybir.dt.float32
BF16 = mybir.dt.bfloat16


def build_module(bench_iters=None):
    nc = bacc.Bacc("TRN2", target_bir_lowering=False, debug=False,
                   num_devices=N_CORES)
    x_ap = nc.dram_tensor("x", [L, D], BF16, kind="ExternalInput").ap()
    w_ap = nc.dram_tensor("w", [D, 3 * HS], F32, kind="ExternalInput").ap()
    mb_ap = nc.dram_tensor("mb", [128, L // 128], F32, kind="ExternalInput").ap()
    out_ap = nc.dram_tensor("out", [LQ, HS], F32, kind="ExternalOutput").ap()

    with tile.TileContext(nc) as tc:
        _build_kernel(tc, x_ap, w_ap, mb_ap, out_ap, bench_iters=bench_iters)
    nc.compile()
    return nc


VARIANT = {"cast_dma": False, "prep_only": False}


def _build_kernel(tc, x_ap, w_ap, mb_ap, out_ap, dbg=None, bench_iters=None):
    from contextlib import ExitStack
    with ExitStack() as ctx:
        _build_kernel_inner(tc, ctx, x_ap, w_ap, mb_ap, out_ap, dbg,
                            bench_iters)


def _build_kernel_inner(tc, ctx, x_ap, w_ap, mb_ap, out_ap, dbg=None,
                        bench_iters=None):
    nc = tc.nc
    DC = D // 128          # 8 d-chunks
    NLG = L // 512         # 8 l-groups of 512 rows
    NQG = LQ // 1024       # 2 query groups
    NKC = L // 128         # 32 key chunks
    W3 = 3 * HS            # 192

    const = ctx.enter_context(tc.tile_pool(name="const", bufs=1))
    xf_pool = ctx.enter_context(tc.tile_pool(name="xf", bufs=3))
    xb_pool = ctx.enter_context(tc.tile_pool(name="xb", bufs=8))
    xt_pool = ctx.enter_context(tc.tile_pool(name="xt", bufs=10))
    e_pool = ctx.enter_context(tc.tile_pool(name="e", bufs=4))
    o_pool = ctx.enter_context(tc.tile_pool(name="o", bufs=3))
    # PSUM: sp (2 banks x 2) shared by x^T-transpose stage and scores stage;
    # qp (1 bank x 2) qkv accum + v_aug transposes; pv (2 banks x 2).
    sp_pool = ctx.enter_context(tc.tile_pool(name="sp", bufs=2, space="PSUM"))
    qp_pool = ctx.enter_context(tc.tile_pool(name="qp", bufs=2, space="PSUM"))
    pv_pool = ctx.enter_context(tc.tile_pool(name="pv", bufs=1, space="PSUM"))

    # ---- constants ----
    wf = const.tile([128, DC * W3], F32)
    for dc in range(DC):
        nc.sync.dma_start(wf[:, dc * W3:(dc + 1) * W3],
                          w_ap[dc * 128:(dc + 1) * 128, :])
    wb = const.tile([128, DC * W3], BF16)
    nc.vector.tensor_copy(wb[:], wf[:])
    mbias = const.tile([128, NKC], F32)
    nc.sync.dma_start(mbias[:], mb_ap[:])
    ident = const.tile([128, 128], BF16)
    make_identity(nc, ident[:])

    qT = const.tile([64, LQ], BF16)
    kT = const.tile([64, L], BF16)
    vT = const.tile([64, L], BF16)
    vaug = const.tile([128, NKC, HS + 1], BF16)
    nc.vector.memset(vaug[:, :, HS:HS + 1], 1.0)

    if bench_iters is not None:
        loop_cm = tc.For_i(0, bench_iters, 1)
        loop_cm.__enter__()

    # attention helpers (emitted interleaved with prep below)
    pv_off = [(qt // 4) * 512 + (qt % 4) * 65 for qt in range(8)]

    def attn_chunk(qg, kc, pv):
        s = sp_pool.tile([128, 1024], F32, tag="sp")
        for half in range(2):
            nc.tensor.matmul(
                s[:, half * 512:(half + 1) * 512],
                lhsT=kT[:, kc * 128:(kc + 1) * 128],
                rhs=qT[:, qg * 1024 + half * 512:
                       qg * 1024 + (half + 1) * 512],
                start=True, stop=True)
        e = e_pool.tile([128, 1024], BF16)
        nc.scalar.activation(e[:], s[:], mybir.ActivationFunctionType.Exp,
                             bias=mbias[:, kc:kc + 1], scale=0.125)
        for qt in range(8):
            # start=True clears has_written for the WHOLE bank: only the
            # first matmul touching each pv bank may set it.
            nc.tensor.matmul(pv[:, pv_off[qt]:pv_off[qt] + 65],
                             lhsT=e[:, qt * 128:(qt + 1) * 128],
                             rhs=vaug[:, kc, :],
                             start=(kc == 0 and qt % 4 == 0),
                             stop=(kc == NKC - 1),
                             skip_group_check=True)

    def attn_norm(qg, pv):
        for qt in range(8):
            r = o_pool.tile([128, 1], F32, tag="r")
            nc.vector.reciprocal(r[:], pv[:, pv_off[qt] + 64:pv_off[qt] + 65])
            o = o_pool.tile([128, HS], F32, tag="o")
            nc.vector.tensor_scalar_mul(o[:], pv[:, pv_off[qt]:pv_off[qt] + 64],
                                        r[:])
            row0 = qg * 1024 + qt * 128
            nc.sync.dma_start(out_ap[row0:row0 + 128, :], o[:])

    pv0 = None

    # ---- phase 1+2: x -> x^T -> qkv^T ----
    for lg in range(NLG):
        xbs = []
        for lt in range(4):
            xb = xb_pool.tile([128, D], BF16)
            rows = slice(lg * 512 + lt * 128, lg * 512 + (lt + 1) * 128)
            nc.sync.dma_start(xb[:], x_ap[rows, :])
            xbs.append(xb)
        # transpose 2 d-chunks per PSUM tile; one wide DVE copy per pair
        xt_sb = []
        for dp in range(DC // 2):
            xtp = sp_pool.tile([128, 1024], BF16, tag="sp")
            for half in range(2):
                dc = dp * 2 + half
                for lt in range(4):
                    nc.tensor.transpose(
                        xtp[:, half * 512 + lt * 128:
                            half * 512 + (lt + 1) * 128],
                        xbs[lt][:, dc * 128:(dc + 1) * 128],
                        ident[:])
            xt = xt_pool.tile([128, 1024], BF16)
            nc.vector.tensor_copy(xt[:], xtp[:])
            xt_sb.append(xt)

        def xt_slice(dc):
            return xt_sb[dc // 2][:, (dc % 2) * 512:(dc % 2 + 1) * 512]

        if lg < NLG // 2:
            # own query half: need q, k, v
            qk_ps = qp_pool.tile([128, 512], F32, tag="qp")
            v_ps = qp_pool.tile([64, 512], F32, tag="qp")
            for dc in range(DC):
                nc.tensor.matmul(qk_ps[:], lhsT=wb[:, dc * W3: dc * W3 + 128],
                                 rhs=xt_slice(dc),
                                 start=(dc == 0), stop=(dc == DC - 1))
            for dc in range(DC):
                nc.tensor.matmul(v_ps[:], lhsT=wb[:, dc * W3 + 128: dc * W3 + 192],
                                 rhs=xt_slice(dc),
                                 start=(dc == 0), stop=(dc == DC - 1))
            sl = slice(lg * 512, (lg + 1) * 512)
            nc.vector.tensor_copy(qT[:, sl], qk_ps[0:64, :])
            nc.vector.tensor_copy(kT[:, sl], qk_ps[64:128, :])
            nc.vector.tensor_copy(vT[:, sl], v_ps[:, :])
        else:
            # other half: only k, v  (W columns 64:192 -> k|v stacked)
            kv_ps = qp_pool.tile([128, 512], F32, tag="qp")
            for dc in range(DC):
                nc.tensor.matmul(kv_ps[:], lhsT=wb[:, dc * W3 + 64: dc * W3 + 192],
                                 rhs=xt_slice(dc),
                                 start=(dc == 0), stop=(dc == DC - 1))
            sl = slice(lg * 512, (lg + 1) * 512)
            nc.vector.tensor_copy(kT[:, sl], kv_ps[0:64, :])
            nc.vector.tensor_copy(vT[:, sl], kv_ps[64:128, :])

        # v_aug chunks for this l-group (keys lg*512 .. +512)
        for kc in range(lg * 4, (lg + 1) * 4):
            vtp = qp_pool.tile([128, 64], BF16, tag="qp")
            nc.tensor.transpose(vtp[:], vT[:, kc * 128:(kc + 1) * 128],
                                ident[0:64, 0:64])
            nc.vector.tensor_copy(vaug[:, kc, 0:HS], vtp[:])

        # interleave qg0 attention over already-resident key chunks so it
        # hides under the remaining l-groups' x DMA
        if lg >= NLG // 2:
            if pv0 is None:
                pv0 = pv_pool.tile([128, 1024], F32, tag="pv")
            for kc in range((lg - 4) * 8, (lg - 3) * 8):
                attn_chunk(0, kc, pv0)

    if dbg is not None:
        nc.gpsimd.dma_start(dbg["qT"][:], qT[:])
        nc.gpsimd.dma_start(dbg["kT"][:], kT[:])
        nc.gpsimd.dma_start(dbg["vT"][:], vT[:])
        nc.gpsimd.dma_start(dbg["vaug"][:], vaug[:].rearrange("p a b -> p (a b)"))

    if VARIANT["prep_only"]:
        # diagnostic: skip attention; just flush something to out
        o = o_pool.tile([128, HS], F32, tag="o")
        nc.vector.tensor_copy(o[:], vaug[:, 0, 0:HS])
        for qt in range(LQ // 128):
            nc.sync.dma_start(out_ap[qt * 128:(qt + 1) * 128, :], o[:])
        if bench_iters is not None:
            loop_cm.__exit__(None, None, None)
        return

    # ---- phase 4: qg0 tail is already emitted; finish qg0 then run qg1 ----
    attn_norm(0, pv0)
    pv1 = pv_pool.tile([128, 1024], F32, tag="pv")
    for kc in range(NKC):
        attn_chunk(1, kc, pv1)
    attn_norm(1, pv1)

    if bench_iters is not None:
        loop_cm.__exit__(None, None, None)


_NC_CACHE = None


def _get_module():
    global _NC_CACHE
    if _NC_CACHE is None:
        _NC_CACHE = build_module()
    return _NC_CACHE


def make_in_maps(x, attn_mask, W_qkv):
    """Host-side sharding: core (b, h) gets x[b] rolled by h*2048 rows."""
    import ml_dtypes
    x = np.asarray(x, dtype=np.float32).astype(ml_dtypes.bfloat16)
    W_qkv = np.ascontiguousarray(np.asarray(W_qkv, dtype=np.float32))
    mask = np.asarray(attn_mask)
    in_maps = []
    for b in range(B):
        for h in range(2):
            if h == 0:
                xr = x[b]
                mr = mask[b]
            else:
                xr = np.concatenate([x[b, LQ:], x[b, :LQ]], axis=0)
                mr = np.concatenate([mask[b, LQ:], mask[b, :LQ]], axis=0)
            bias = np.where(mr, 0.0, MASK_NEG).astype(np.float32)
            mb = np.ascontiguousarray(bias.reshape(L // 128, 128).T)
            in_maps.append({"x": np.ascontiguousarray(xr),
                            "w": W_qkv, "mb": mb})
    return in_maps


def _pos2l():
    """Device position i = b*512 + j*128 + p  <->  row l = b*512 + 4p + j
    (from the 4-rows-per-partition DMA layout)."""
    b = np.arange(L // 512)[:, None, None]
    j = np.arange(4)[None, :, None]
    p = np.arange(128)[None, None, :]
    return (b * 512 + 4 * p + j).reshape(-1)


def assemble_out(results):
    out = np.empty((B, L, HS), dtype=np.float32)
    for b in range(B):
        for h in range(2):
            out[b, h * LQ:(h + 1) * LQ] = results[b * 2 + h]["out"]
    return out


def kernel(x, attn_mask, W_qkv):
    nc = _get_module()
    in_maps = make_in_maps(x, attn_mask, W_qkv)
    res = run_bass_kernel_spmd(nc, in_maps, core_ids=list(range(N_CORES)))
    return assemble_out(res.results)
